# revision 34
# baseline (speedup 1.0000x reference)
"""Trainium2 Bass kernel for DifferentiableKMeans retrieval-knn.

Computes, for x [32768, 128] and cluster_centers [512, 128]:
    d2 = ||x||^2 - 2 x@c.T + ||c||^2          [N, 512]
    idx = top-10 smallest-distance cluster ids  [N, 10]
    out = x[idx].reshape(1, N*10, 128)          (gather of x rows 0..511)

Sharding: data-parallel over N across 8 NeuronCores; centers and the
gather table (x[:512]) replicated.

Per-core pipeline (4096 points, 32 tiles of 128):
  PE:  transpose(x_tile), fp32 matmul accumulated onto an ACT-preloaded
       -0.5|c|^2 PSUM bias image.
  DVE: max8 / max_index / match_replace / max8 / max_index => top-10 ids.
  Gather is split to balance engines (the Q7 SWDGE descriptor generation
  runs ~8.8ns/row and would otherwise dominate):
    ranks 0-5: gpsimd dma_gather (16-wrapped int16 index list, built for
               4 tiles at a time on the PE to amortize fixed matmul costs)
    ranks 6-9: PE one-hot gather - idx column transposed+broadcast, DVE
               integer is_equal against iota builds a bf16 one-hot lhsT,
               4 accumulating bf16 matmuls against the SBUF-resident bf16
               table produce the gathered rows (bf16-rounded, within the
               2e-2 gate).
  One contiguous 655KB store per tile into the final output layout.

Ranking is by m = x.c - 0.5*|c|^2 (monotone equivalent of distance per row).
"""

import os
import sys

for _p in ("/opt/trn_rl_repo", "/root/.axon_site/_ro/trn_rl_repo"):
    if os.path.isdir(_p) and _p not in sys.path:
        sys.path.insert(0, _p)

import numpy as np

N_FULL = 32768
D = 128
K = 512
TOPK = 10
N_CORES = 8
N_SHARD = N_FULL // N_CORES  # 4096
P = 128

Q_RANKS = 6                 # ranks gathered via gpsimd dma_gather
E_RANKS = TOPK - Q_RANKS    # ranks gathered via PE one-hot matmul
WGROUP = 2                  # tiles per batched wrap build

_BUILD_CACHE = {}


def build_nc(n_points=N_SHARD):
    key = (n_points, Q_RANKS)
    if key in _BUILD_CACHE:
        return _BUILD_CACHE[key]

    import concourse.bass as bass
    import concourse.mybir as mybir
    from concourse import bacc
    from concourse.masks import make_identity
    from concourse.tile import TileContext

    f32 = mybir.dt.float32
    bf16 = mybir.dt.bfloat16
    AFT = mybir.ActivationFunctionType
    nt = n_points // P
    assert n_points % P == 0 and nt % WGROUP == 0

    nc = bacc.Bacc("TRN2", target_bir_lowering=False, debug=False,
                  num_swdge_queues=4)

    x = nc.dram_tensor("x", [n_points, D], f32, kind="ExternalInput")
    xhead = nc.dram_tensor("xhead", [K, D], f32, kind="ExternalInput")
    cc = nc.dram_tensor("cluster_centers", [K, D], f32, kind="ExternalInput")
    out = nc.dram_tensor("out", [n_points * TOPK, D], f32, kind="ExternalOutput")

    NW = Q_RANKS * 8          # wrapped idx cols per tile
    with TileContext(nc) as tc:
        with tc.tile_pool(name="const", bufs=1) as const_pool:
            identity = const_pool.tile([P, P], f32)
            make_identity(nc, identity[:])

            cT = const_pool.tile([P, K], f32)          # centers transposed [d, k]
            negc2 = const_pool.tile([1, K], f32)       # -0.5*|c_k|^2 row
            negc2f = const_pool.tile([P, K], f32)      # bias bcast to 128 parts
            ones_row = const_pool.tile([1, P], f32)
            nc.vector.memset(ones_row[:], 1.0)
            c2col = const_pool.tile([P, K // P], f32)

            # bf16 copy of the gather table, 4 chunks of [128, 128]
            xh16 = []
            for c in range(4):
                t16 = const_pool.tile([P, P], bf16, tag=f"xh16_{c}")
                xh16.append(t16)

            # iota columns (partition index + 128*c) for the one-hot compare
            iota_i = const_pool.tile([P, 1], mybir.dt.int32)
            nc.gpsimd.iota(iota_i[:], pattern=[[0, 1]], base=0,
                           channel_multiplier=1)
            iota_f = const_pool.tile([P, 4], f32)

            # REP[k, m] = 1 iff m % 16 == k (replicates to the 8 Q7 blocks)
            rep = const_pool.tile([16, P], f32)
            rep3 = rep[:].rearrange("k (u l) -> k u l", l=16)
            nc.gpsimd.memset(rep3, 0.0)
            nc.gpsimd.affine_select(
                out=rep3, in_=rep3,
                compare_op=mybir.AluOpType.not_equal, fill=1.0,
                base=0, pattern=[[0, P // 16], [-1, 16]], channel_multiplier=1,
            )

            with tc.tile_pool(name="prep", bufs=2) as prep_pool, \
                 tc.tile_pool(name="prep_ps", bufs=2, space="PSUM") as prep_psum:
                for c in range(4):
                    nc.scalar.activation(iota_f[:, c:c + 1], iota_i[:],
                                         AFT.Copy, bias=float(P * c))
                c2row_ps = prep_psum.tile([1, K], f32, tag="c2row")
                for j in range(K // P):
                    cchunk = prep_pool.tile([P, P], f32, tag="cchunk")
                    nc.sync.dma_start(cchunk[:], cc[j * P:(j + 1) * P, :])
                    pst = prep_psum.tile([P, P], f32, tag="pst")
                    nc.tensor.transpose(pst[:], cchunk[:], identity[:])
                    nc.scalar.copy(cT[:, j * P:(j + 1) * P], pst[:])
                    sq = prep_pool.tile([P, P], f32, tag="sq")
                    nc.scalar.activation(
                        sq[:], cchunk[:], AFT.Square,
                        accum_out=c2col[:, j:j + 1],
                    )
                    nc.tensor.matmul(
                        c2row_ps[0:1, j * P:(j + 1) * P],
                        lhsT=c2col[:, j:j + 1], rhs=identity[:],
                        start=True, stop=True,
                    )
                    # bf16 gather table chunk
                    xchunk = prep_pool.tile([P, P], f32, tag="xchunk")
                    nc.sync.dma_start(xchunk[:], xhead[j * P:(j + 1) * P, :])
                    nc.scalar.copy(xh16[j][:], xchunk[:])
                nc.scalar.activation(negc2[:], c2row_ps[:], AFT.Copy, scale=-0.5)
                negc2f_ps = prep_psum.tile([P, K], f32, tag="negc2f")
                nc.tensor.matmul(negc2f_ps[:], lhsT=ones_row[:], rhs=negc2[:],
                                 start=True, stop=True)
                nc.scalar.copy(negc2f[:], negc2f_ps[:])

            with tc.tile_pool(name="xin", bufs=4) as xin_pool, \
                 tc.tile_pool(name="xt", bufs=4) as xt_pool, \
                 tc.tile_pool(name="ms", bufs=4) as ms_pool, \
                 tc.tile_pool(name="ms2", bufs=4) as ms2_pool, \
                 tc.tile_pool(name="small", bufs=6) as small_pool, \
                 tc.tile_pool(name="wrapb", bufs=2) as wrap_pool, \
                 tc.tile_pool(name="oh", bufs=5) as oh_pool, \
                 tc.tile_pool(name="gath", bufs=8) as gath_pool, \
                 tc.tile_pool(name="mm_ps", bufs=2, space="PSUM") as mm_psum, \
                 tc.tile_pool(name="wrap_ps", bufs=1, space="PSUM") as wrap_psum, \
                 tc.tile_pool(name="g_ps", bufs=2, space="PSUM") as g_psum, \
                 tc.tile_pool(name="bt_ps", bufs=2, space="PSUM") as bt_psum, \
                 tc.tile_pool(name="tr_ps", bufs=1, space="PSUM") as tr_psum:
                idxf4 = None
                widx4 = None
                group = []  # (g_tile, tile_idx) pending gathers this group
                for i in range(nt):
                    t = i % WGROUP
                    if t == 0:
                        idxf4 = wrap_pool.tile([P, WGROUP * Q_RANKS], f32,
                                               tag="idxf4")
                        group = []

                    x_tile = xin_pool.tile([P, D], f32, tag="x")
                    nc.sync.dma_start(x_tile[:], x[i * P:(i + 1) * P, :])

                    pst = tr_psum.tile([P, P], f32, tag="xtp")
                    nc.tensor.transpose(pst[:], x_tile[:], identity[:])
                    xT = xt_pool.tile([P, P], f32, tag="xT")
                    nc.scalar.copy(xT[:], pst[:])

                    pm = mm_psum.tile([P, K], f32, tag="pm")
                    if i < 2:
                        # first use of each PSUM buf after the prep matmuls:
                        # the ACT-preload + accumulate form misreads the
                        # first partition quad on hardware, so pay for the
                        # explicit rank-1 bias matmul here.
                        nc.tensor.matmul(pm[:], lhsT=xT[:], rhs=cT[:],
                                         start=True, stop=False)
                        nc.tensor.matmul(pm[:], lhsT=ones_row[:], rhs=negc2[:],
                                         start=False, stop=True)
                    else:
                        nc.scalar.copy(pm[:], negc2f[:])
                        nc.tensor.matmul(pm[:], lhsT=xT[:], rhs=cT[:],
                                         start=False, stop=True)

                    ms = ms_pool.tile([P, K], f32, tag="ms")
                    nc.scalar.copy(ms[:], pm[:])

                    v8 = small_pool.tile([P, 8], f32, tag="v8")
                    nc.vector.max(v8[:], ms[:])
                    idx = small_pool.tile([P, 16], mybir.dt.uint32, tag="idx")
                    nc.vector.max_index(idx[:, 0:8], v8[:], ms[:])
                    ms2 = ms2_pool.tile([P, K], f32, tag="ms2")
                    nc.vector.match_replace(
                        out=ms2[:], in_to_replace=v8[:], in_values=ms[:],
                        imm_value=-1e30,
                    )
                    v8b = small_pool.tile([P, 8], f32, tag="v8b")
                    nc.vector.max(v8b[:], ms2[:])
                    nc.vector.max_index(idx[:, 8:16], v8b[:], ms2[:])

                    idx_f = small_pool.tile([P, TOPK], f32, tag="idxf")
                    nc.scalar.copy(idx_f[:], idx[:, 0:TOPK])


                    nc.scalar.copy(idxf4[:, t * Q_RANKS:(t + 1) * Q_RANKS],
                                   idx_f[:, 0:Q_RANKS])
                    g = gath_pool.tile([P, TOPK * D], f32, tag="g")

                    # ---- PE one-hot gather for ranks Q_RANKS..TOPK ----
                    # bcT4[k', e*P + j] = idx[j, Q_RANKS + e]
                    bcT4 = oh_pool.tile([P, E_RANKS * P], mybir.dt.float16,
                                        tag="bcT4")
                    for e in range(E_RANKS):
                        # single-column tile (offset 0) - the weight-load
                        # transpose mishandles broadcast APs with a nonzero
                        # free offset on a pitched tile
                        col1 = small_pool.tile([P, 1], f32, tag=f"col{e}")
                        nc.scalar.copy(col1[:],
                                       idx_f[:, Q_RANKS + e:Q_RANKS + e + 1])
                        bt_ps = bt_psum.tile([P, P], f32, tag="bt")
                        nc.tensor.transpose(
                            bt_ps[:], col1[:].to_broadcast([P, P]),
                            identity[:])
                        nc.scalar.copy(bcT4[:, e * P:(e + 1) * P], bt_ps[:])
                    # one is_equal per table chunk covering all PE ranks
                    ohs = []
                    for c in range(4):
                        oh = oh_pool.tile([P, E_RANKS * P], bf16, tag=f"oh{c}")
                        nc.vector.tensor_scalar(
                            out=oh[:], in0=bcT4[:],
                            scalar1=iota_f[:, c:c + 1], scalar2=None,
                            op0=mybir.AluOpType.is_equal,
                        )
                        ohs.append(oh)
                    for e in range(E_RANKS):
                        r = Q_RANKS + e
                        gp = g_psum.tile([P, P], f32, tag="gp")
                        for c in range(4):
                            nc.tensor.matmul(
                                gp[:], lhsT=ohs[c][:, e * P:(e + 1) * P],
                                rhs=xh16[c][:], start=(c == 0), stop=(c == 3))
                        nc.scalar.copy(g[:, r * D:(r + 1) * D], gp[:])

                    group.append((g, i))

                    if t == WGROUP - 1:
                        # ---- batched 16-wrap build for Q7 ranks ----
                        # one shared PSUM bank, stages serialized by reuse
                        nb = WGROUP * Q_RANKS
                        wf = max(8 * nb, P)
                        wps = wrap_psum.tile([P, wf], f32, tag="wps")
                        nc.tensor.transpose(wps[0:nb, 0:P], idxf4[:],
                                            identity[:])
                        t_sb = wrap_pool.tile([nb, P], f32, tag="t_sb")
                        nc.scalar.copy(t_sb[:], wps[0:nb, 0:P])
                        wps2 = wrap_psum.tile([P, wf], f32, tag="wps")
                        for u in range(8):
                            nc.tensor.transpose(
                                wps2[0:16, u:8 * nb:8],
                                t_sb[:, 16 * u:16 * (u + 1)],
                                identity[0:nb, 0:nb],
                            )
                        w16_sb = wrap_pool.tile([16, 8 * nb], f32, tag="w16sb")
                        nc.scalar.copy(w16_sb[:], wps2[0:16, 0:8 * nb])
                        wps3 = wrap_psum.tile([P, wf], f32, tag="wps")
                        nc.tensor.matmul(wps3[:, 0:8 * nb], lhsT=rep[:],
                                         rhs=w16_sb[:], start=True, stop=True)
                        widx4 = wrap_pool.tile([P, 8 * nb], mybir.dt.int16,
                                               tag="widx4")
                        nc.scalar.copy(widx4[:], wps3[:, 0:8 * nb])

                        for gt, gi in group:
                            tt = gi % WGROUP
                            nc.gpsimd.dma_gather(
                                out_ap=gt[:, 0:Q_RANKS * D].rearrange(
                                    "p (r d) -> p r d", r=Q_RANKS),
                                in_ap=xhead[:],
                                idxs_ap=widx4[:, tt * NW:(tt + 1) * NW],
                                num_idxs=P * Q_RANKS,
                                num_idxs_reg=P * Q_RANKS,
                                elem_size=D,
                                single_packet=False,
                                queue_num=gi % 4,
                            )
                            out_view = out[gi * P * TOPK:(gi + 1) * P * TOPK,
                                           :].rearrange("(p r) d -> p (r d)",
                                                        p=P)
                            nc.sync.dma_start(out_view, gt[:])

    nc.compile()
    _BUILD_CACHE[key] = nc
    return nc


def run_on_cores(x_np, cc_np, trace=False):
    """Run the SPMD kernel on all 8 cores. Returns (out [N*10,D], results)."""
    from concourse import bass_utils

    nc = build_nc(N_SHARD)
    xhead = np.ascontiguousarray(x_np[:K])
    in_maps = [
        {
            "x": np.ascontiguousarray(x_np[c * N_SHARD:(c + 1) * N_SHARD]),
            "xhead": xhead,
            "cluster_centers": cc_np,
        }
        for c in range(N_CORES)
    ]
    res = bass_utils.run_bass_kernel_spmd(
        nc, in_maps, core_ids=list(range(N_CORES)), trace=trace,
    )
    shards = [res.results[c]["out"] for c in range(N_CORES)]
    full = np.concatenate(shards, axis=0)  # [N*10, D]
    return full, res


def kernel(x, cluster_centers):
    x_np = np.ascontiguousarray(np.asarray(x, dtype=np.float32))
    cc_np = np.ascontiguousarray(np.asarray(cluster_centers, dtype=np.float32))
    full, _ = run_on_cores(x_np, cc_np, trace=False)
    return full.reshape(1, N_FULL * TOPK, D)


# revision 35
# speedup vs baseline: 1.0652x; 1.0652x over previous
"""Trainium2 Bass kernel for DifferentiableKMeans retrieval-knn.

Computes, for x [32768, 128] and cluster_centers [512, 128]:
    d2 = ||x||^2 - 2 x@c.T + ||c||^2          [N, 512]
    idx = top-10 smallest-distance cluster ids  [N, 10]
    out = x[idx].reshape(1, N*10, 128)          (gather of x rows 0..511)

Sharding: data-parallel over N across 8 NeuronCores; centers and the
gather table (x[:512]) replicated.

Per-core pipeline (4096 points, 32 tiles of 128):
  PE:  transpose(x_tile), fp32 matmul accumulated onto an ACT-preloaded
       -0.5|c|^2 PSUM bias image.
  DVE: max8 / max_index / match_replace / max8 / max_index => top-10 ids.
  Gather is split to balance engines (the Q7 SWDGE descriptor generation
  runs ~8.8ns/row and would otherwise dominate):
    ranks 0-5: gpsimd dma_gather (16-wrapped int16 index list, built for
               4 tiles at a time on the PE to amortize fixed matmul costs)
    ranks 6-9: PE one-hot gather - idx column transposed+broadcast, DVE
               integer is_equal against iota builds a bf16 one-hot lhsT,
               4 accumulating bf16 matmuls against the SBUF-resident bf16
               table produce the gathered rows (bf16-rounded, within the
               2e-2 gate).
  One contiguous 655KB store per tile into the final output layout.

Ranking is by m = x.c - 0.5*|c|^2 (monotone equivalent of distance per row).
"""

import os
import sys

for _p in ("/opt/trn_rl_repo", "/root/.axon_site/_ro/trn_rl_repo"):
    if os.path.isdir(_p) and _p not in sys.path:
        sys.path.insert(0, _p)

import numpy as np

N_FULL = 32768
D = 128
K = 512
TOPK = 10
N_CORES = 8
N_SHARD = N_FULL // N_CORES  # 4096
P = 128

Q_RANKS = 6                 # ranks gathered via gpsimd dma_gather
E_RANKS = TOPK - Q_RANKS    # ranks gathered via PE one-hot matmul
WGROUP = 1                  # tiles per batched wrap build

_BUILD_CACHE = {}


def build_nc(n_points=N_SHARD):
    key = (n_points, Q_RANKS)
    if key in _BUILD_CACHE:
        return _BUILD_CACHE[key]

    import concourse.bass as bass
    import concourse.mybir as mybir
    from concourse import bacc
    from concourse.masks import make_identity
    from concourse.tile import TileContext

    f32 = mybir.dt.float32
    bf16 = mybir.dt.bfloat16
    AFT = mybir.ActivationFunctionType
    nt = n_points // P
    assert n_points % P == 0 and nt % WGROUP == 0

    nc = bacc.Bacc("TRN2", target_bir_lowering=False, debug=False,
                  num_swdge_queues=4)

    x = nc.dram_tensor("x", [n_points, D], f32, kind="ExternalInput")
    xhead = nc.dram_tensor("xhead", [K, D], f32, kind="ExternalInput")
    cc = nc.dram_tensor("cluster_centers", [K, D], f32, kind="ExternalInput")
    out = nc.dram_tensor("out", [n_points * TOPK, D], f32, kind="ExternalOutput")

    NW = Q_RANKS * 8          # wrapped idx cols per tile
    with TileContext(nc) as tc:
        with tc.tile_pool(name="const", bufs=1) as const_pool:
            identity = const_pool.tile([P, P], f32)
            make_identity(nc, identity[:])

            cT = const_pool.tile([P, K], f32)          # centers transposed [d, k]
            negc2 = const_pool.tile([1, K], f32)       # -0.5*|c_k|^2 row
            negc2f = const_pool.tile([P, K], f32)      # bias bcast to 128 parts
            ones_row = const_pool.tile([1, P], f32)
            nc.vector.memset(ones_row[:], 1.0)
            c2col = const_pool.tile([P, K // P], f32)

            # bf16 copy of the gather table, 4 chunks of [128, 128]
            xh16 = []
            for c in range(4):
                t16 = const_pool.tile([P, P], bf16, tag=f"xh16_{c}")
                xh16.append(t16)

            # iota columns (partition index + 128*c) for the one-hot compare
            iota_i = const_pool.tile([P, 1], mybir.dt.int32)
            nc.gpsimd.iota(iota_i[:], pattern=[[0, 1]], base=0,
                           channel_multiplier=1)
            iota_f = const_pool.tile([P, 4], f32)

            # REP[k, m] = 1 iff m % 16 == k (replicates to the 8 Q7 blocks)
            rep = const_pool.tile([16, P], f32)
            rep3 = rep[:].rearrange("k (u l) -> k u l", l=16)
            nc.gpsimd.memset(rep3, 0.0)
            nc.gpsimd.affine_select(
                out=rep3, in_=rep3,
                compare_op=mybir.AluOpType.not_equal, fill=1.0,
                base=0, pattern=[[0, P // 16], [-1, 16]], channel_multiplier=1,
            )

            with tc.tile_pool(name="prep", bufs=2) as prep_pool, \
                 tc.tile_pool(name="prep_ps", bufs=2, space="PSUM") as prep_psum:
                for c in range(4):
                    nc.scalar.activation(iota_f[:, c:c + 1], iota_i[:],
                                         AFT.Copy, bias=float(P * c))
                c2row_ps = prep_psum.tile([1, K], f32, tag="c2row")
                for j in range(K // P):
                    cchunk = prep_pool.tile([P, P], f32, tag="cchunk")
                    nc.sync.dma_start(cchunk[:], cc[j * P:(j + 1) * P, :])
                    pst = prep_psum.tile([P, P], f32, tag="pst")
                    nc.tensor.transpose(pst[:], cchunk[:], identity[:])
                    nc.scalar.copy(cT[:, j * P:(j + 1) * P], pst[:])
                    sq = prep_pool.tile([P, P], f32, tag="sq")
                    nc.scalar.activation(
                        sq[:], cchunk[:], AFT.Square,
                        accum_out=c2col[:, j:j + 1],
                    )
                    nc.tensor.matmul(
                        c2row_ps[0:1, j * P:(j + 1) * P],
                        lhsT=c2col[:, j:j + 1], rhs=identity[:],
                        start=True, stop=True,
                    )
                    # bf16 gather table chunk
                    xchunk = prep_pool.tile([P, P], f32, tag="xchunk")
                    nc.sync.dma_start(xchunk[:], xhead[j * P:(j + 1) * P, :])
                    nc.scalar.copy(xh16[j][:], xchunk[:])
                nc.scalar.activation(negc2[:], c2row_ps[:], AFT.Copy, scale=-0.5)
                negc2f_ps = prep_psum.tile([P, K], f32, tag="negc2f")
                nc.tensor.matmul(negc2f_ps[:], lhsT=ones_row[:], rhs=negc2[:],
                                 start=True, stop=True)
                nc.scalar.copy(negc2f[:], negc2f_ps[:])

            with tc.tile_pool(name="xin", bufs=4) as xin_pool, \
                 tc.tile_pool(name="xt", bufs=4) as xt_pool, \
                 tc.tile_pool(name="ms", bufs=4) as ms_pool, \
                 tc.tile_pool(name="ms2", bufs=4) as ms2_pool, \
                 tc.tile_pool(name="small", bufs=6) as small_pool, \
                 tc.tile_pool(name="wrapb", bufs=2) as wrap_pool, \
                 tc.tile_pool(name="oh", bufs=5) as oh_pool, \
                 tc.tile_pool(name="gath", bufs=8) as gath_pool, \
                 tc.tile_pool(name="mm_ps", bufs=2, space="PSUM") as mm_psum, \
                 tc.tile_pool(name="wrap_ps", bufs=1, space="PSUM") as wrap_psum, \
                 tc.tile_pool(name="g_ps", bufs=2, space="PSUM") as g_psum, \
                 tc.tile_pool(name="bt_ps", bufs=2, space="PSUM") as bt_psum, \
                 tc.tile_pool(name="tr_ps", bufs=1, space="PSUM") as tr_psum:
                idxf4 = None
                widx4 = None
                group = []  # (g_tile, tile_idx) pending gathers this group
                for i in range(nt):
                    t = i % WGROUP
                    if t == 0:
                        idxf4 = wrap_pool.tile([P, WGROUP * Q_RANKS], f32,
                                               tag="idxf4")
                        group = []

                    x_tile = xin_pool.tile([P, D], f32, tag="x")
                    nc.sync.dma_start(x_tile[:], x[i * P:(i + 1) * P, :])

                    pst = tr_psum.tile([P, P], f32, tag="xtp")
                    nc.tensor.transpose(pst[:], x_tile[:], identity[:])
                    xT = xt_pool.tile([P, P], f32, tag="xT")
                    nc.scalar.copy(xT[:], pst[:])

                    pm = mm_psum.tile([P, K], f32, tag="pm")
                    if i < 2:
                        # first use of each PSUM buf after the prep matmuls:
                        # the ACT-preload + accumulate form misreads the
                        # first partition quad on hardware, so pay for the
                        # explicit rank-1 bias matmul here.
                        nc.tensor.matmul(pm[:], lhsT=xT[:], rhs=cT[:],
                                         start=True, stop=False)
                        nc.tensor.matmul(pm[:], lhsT=ones_row[:], rhs=negc2[:],
                                         start=False, stop=True)
                    else:
                        nc.scalar.copy(pm[:], negc2f[:])
                        nc.tensor.matmul(pm[:], lhsT=xT[:], rhs=cT[:],
                                         start=False, stop=True)

                    ms = ms_pool.tile([P, K], f32, tag="ms")
                    nc.scalar.copy(ms[:], pm[:])

                    v8 = small_pool.tile([P, 8], f32, tag="v8")
                    nc.vector.max(v8[:], ms[:])
                    idx = small_pool.tile([P, 16], mybir.dt.uint32, tag="idx")
                    nc.vector.max_index(idx[:, 0:8], v8[:], ms[:])
                    ms2 = ms2_pool.tile([P, K], f32, tag="ms2")
                    nc.vector.match_replace(
                        out=ms2[:], in_to_replace=v8[:], in_values=ms[:],
                        imm_value=-1e30,
                    )
                    v8b = small_pool.tile([P, 8], f32, tag="v8b")
                    nc.vector.max(v8b[:], ms2[:])
                    nc.vector.max_index(idx[:, 8:16], v8b[:], ms2[:])

                    idx_f = small_pool.tile([P, TOPK], f32, tag="idxf")
                    nc.scalar.copy(idx_f[:], idx[:, 0:TOPK])


                    g = gath_pool.tile([P, TOPK * D], f32, tag="g")

                    # ---- PE one-hot gather for ranks Q_RANKS..TOPK ----
                    # bcT4[k', e*P + j] = idx[j, Q_RANKS + e]
                    bcT4 = oh_pool.tile([P, E_RANKS * P], mybir.dt.float16,
                                        tag="bcT4")
                    for e in range(E_RANKS):
                        # single-column tile (offset 0) - the weight-load
                        # transpose mishandles broadcast APs with a nonzero
                        # free offset on a pitched tile
                        col1 = small_pool.tile([P, 1], f32, tag=f"col{e}")
                        nc.scalar.copy(col1[:],
                                       idx_f[:, Q_RANKS + e:Q_RANKS + e + 1])
                        bt_ps = bt_psum.tile([P, P], f32, tag="bt")
                        nc.tensor.transpose(
                            bt_ps[:], col1[:].to_broadcast([P, P]),
                            identity[:])
                        nc.scalar.copy(bcT4[:, e * P:(e + 1) * P], bt_ps[:])
                    # one is_equal per table chunk covering all PE ranks
                    ohs = []
                    for c in range(4):
                        oh = oh_pool.tile([P, E_RANKS * P], bf16, tag=f"oh{c}")
                        nc.vector.tensor_scalar(
                            out=oh[:], in0=bcT4[:],
                            scalar1=iota_f[:, c:c + 1], scalar2=None,
                            op0=mybir.AluOpType.is_equal,
                        )
                        ohs.append(oh)
                    for e in range(E_RANKS):
                        r = Q_RANKS + e
                        gp = g_psum.tile([P, P], f32, tag="gp")
                        for c in range(4):
                            nc.tensor.matmul(
                                gp[:], lhsT=ohs[c][:, e * P:(e + 1) * P],
                                rhs=xh16[c][:], start=(c == 0), stop=(c == 3))
                        nc.scalar.copy(g[:, r * D:(r + 1) * D], gp[:])

                    group.append((g, i))

                    if t == WGROUP - 1:
                        # ---- batched 16-wrap build for Q7 ranks ----
                        # one shared PSUM bank, stages serialized by reuse
                        nb = WGROUP * Q_RANKS
                        wf = max(8 * nb, P)
                        wps = wrap_psum.tile([P, wf], f32, tag="wps")
                        nc.tensor.transpose(wps[0:nb, 0:P],
                                            idx_f[:, 0:Q_RANKS], identity[:])
                        t_sb = wrap_pool.tile([nb, P], f32, tag="t_sb")
                        nc.scalar.copy(t_sb[:], wps[0:nb, 0:P])
                        wps2 = wrap_psum.tile([P, wf], f32, tag="wps")
                        for u in range(8):
                            nc.tensor.transpose(
                                wps2[0:16, u:8 * nb:8],
                                t_sb[:, 16 * u:16 * (u + 1)],
                                identity[0:nb, 0:nb],
                            )
                        w16_sb = wrap_pool.tile([16, 8 * nb], f32, tag="w16sb")
                        nc.scalar.copy(w16_sb[:], wps2[0:16, 0:8 * nb])
                        wps3 = wrap_psum.tile([P, wf], f32, tag="wps")
                        nc.tensor.matmul(wps3[:, 0:8 * nb], lhsT=rep[:],
                                         rhs=w16_sb[:], start=True, stop=True)
                        widx4 = wrap_pool.tile([P, 8 * nb], mybir.dt.int16,
                                               tag="widx4")
                        nc.scalar.copy(widx4[:], wps3[:, 0:8 * nb])

                        for gt, gi in group:
                            tt = gi % WGROUP
                            nc.gpsimd.dma_gather(
                                out_ap=gt[:, 0:Q_RANKS * D].rearrange(
                                    "p (r d) -> p r d", r=Q_RANKS),
                                in_ap=xhead[:],
                                idxs_ap=widx4[:, tt * NW:(tt + 1) * NW],
                                num_idxs=P * Q_RANKS,
                                num_idxs_reg=P * Q_RANKS,
                                elem_size=D,
                                single_packet=False,
                                queue_num=gi % 4,
                            )
                            out_view = out[gi * P * TOPK:(gi + 1) * P * TOPK,
                                           :].rearrange("(p r) d -> p (r d)",
                                                        p=P)
                            nc.sync.dma_start(out_view, gt[:])

    nc.compile()
    _BUILD_CACHE[key] = nc
    return nc


def run_on_cores(x_np, cc_np, trace=False):
    """Run the SPMD kernel on all 8 cores. Returns (out [N*10,D], results)."""
    from concourse import bass_utils

    nc = build_nc(N_SHARD)
    xhead = np.ascontiguousarray(x_np[:K])
    in_maps = [
        {
            "x": np.ascontiguousarray(x_np[c * N_SHARD:(c + 1) * N_SHARD]),
            "xhead": xhead,
            "cluster_centers": cc_np,
        }
        for c in range(N_CORES)
    ]
    res = bass_utils.run_bass_kernel_spmd(
        nc, in_maps, core_ids=list(range(N_CORES)), trace=trace,
    )
    shards = [res.results[c]["out"] for c in range(N_CORES)]
    full = np.concatenate(shards, axis=0)  # [N*10, D]
    return full, res


def kernel(x, cluster_centers):
    x_np = np.ascontiguousarray(np.asarray(x, dtype=np.float32))
    cc_np = np.ascontiguousarray(np.asarray(cluster_centers, dtype=np.float32))
    full, _ = run_on_cores(x_np, cc_np, trace=False)
    return full.reshape(1, N_FULL * TOPK, D)
